# revision 73
# baseline (speedup 1.0000x reference)
"""Trainium2 Bass kernel for conv-projected multi-head attention.

Reference computation (per batch element b of 8):
  q  = conv1x1(x, Wq)                     # [512, 32, 32]
  kv = conv3x3(x, Wkv, pad=1)             # [1024, 32, 32] -> k, v
  per head h (8 heads, d=64): attn = softmax(q k^T / sqrt(d)); o = attn v
  out = conv1x1(gelu(o), Wout) + bout     # [256, 32, 32]

Sharding: data-parallel over batch. Core b computes batch element b
end-to-end; no collectives.

Design notes (the cost model charges matmuls by OUTPUT FREE SIZE per
K<=128 slab; fp8 with DoubleRow halves that and doubles K per slab):
  - q/k/v convs run in fp8 (e4m3) DoubleRow with a 3-pass hi/lo
    decomposition: W'x ~ W_hi x_hi + W_hi x_lo + W_lo x_hi, with
    W' = 16W so the fp8 exponent range is used well. That is 0.75x the
    bf16 matmul cost at bf16-level accuracy (the dropped lo*lo term is
    ~0.1%). The 16x scales cancel inside the exp scale (q'k' = 256 qk)
    and via a 16-valued ones column for the attn@v denominator.
  - v conv output [ch, pix] is PE-transposed (identity matmul) into
    per-(jc, mi) vaug tiles [j, 2 heads, 65]; conv psum units are
    jc-pair sized so each drain->transpose chain hides under the next
    unit's matmuls.
  - dots e^T[j, i] = exp(scale' k q^T) per head, bf16, interleaved
    into the conv phase (~12 fp8 conv matmuls per dots psum tile) so
    the Scalar engine's exp stream (~68us) hides under PE conv work.
    All 64 e^T tiles stay live in SBUF (128KB/partition; weight pools
    release before the attention phase opens its pools).
  - attn@v outputs land [i, 65] per 128-query chunk (free 65 instead
    of 1024): po[i, :64] = unnormalized out, po[i, 64] = denominator.
  - normalization fuses into the gelu's per-partition scale AP
    (DVE reciprocal of the denominator column); the gelu reads psum
    directly (GPSIMD must not touch PSUM on real hardware).
  - gg is PE-transposed back to [d, i] for the 1x1 output projection.
  - the attention phase is software-pipelined per 128-query column
    chunk: attnv(ic) | transpose(ic-1) | proj+store(ic-2).
"""

import sys
from contextlib import ExitStack

import numpy as np

sys.path.insert(0, "/opt/trn_rl_repo")

import ml_dtypes  # noqa: E402
import concourse.bass as bass  # noqa: E402
import concourse.tile as tile  # noqa: E402
from concourse import bacc, mybir  # noqa: E402
from concourse.bass_utils import run_bass_kernel_spmd  # noqa: E402

BF16 = ml_dtypes.bfloat16

B, C, H, W = 8, 256, 32, 32
HEADS, D = 8, 64
INNER = HEADS * D  # 512
N = H * W  # 1024
SCALE = D ** -0.5
HP, WP = H + 2, W + 2  # padded image
NCORES = 8

dt = mybir.dt
AF = mybir.ActivationFunctionType


def emit(tc, ins, out_ap):
    nc = tc.nc
    ctx = tc._emit_ctx  # ExitStack owned by caller

    consts = ctx.enter_context(tc.tile_pool(name="consts", bufs=1))
    vspool = tc.alloc_tile_pool(name="vspool", bufs=2)
    wpool = tc.alloc_tile_pool(name="wpool", bufs=1)

    # ---- weight/input loads, smallest/earliest-needed first ----
    # fp8 hi/lo weight + image tensors (hl: 0=hi, 1=lo)
    qtap_sb = consts.tile([128, 2, 2, 512], dt.float8e4, name="qtap_sb")
    qtap_v = ins["wq"].rearrange("p (l c m) -> p l c m", l=2, c=2, m=512)
    nc.sync.dma_start(qtap_sb[:, 0:1], qtap_v[:, 0:1])
    nc.gpsimd.dma_start(qtap_sb[:, 1:2], qtap_v[:, 1:2])
    xp_sb = wpool.tile([128, 2, 2, HP * WP], dt.float8e4, name="xp_sb")
    xp_v = ins["xp"].rearrange("p (l c n) -> p l c n", l=2, c=2, n=HP * WP)
    nc.sync.dma_start(xp_sb[:, 0:1], xp_v[:, 0:1])
    nc.gpsimd.dma_start(xp_sb[:, 1:2], xp_v[:, 1:2])
    # k taps packed per out-chunk g: [128, hl, g, c2, t, 128]
    wk_sb = wpool.tile([128, 2, 4, 2, 9, 128], dt.float8e4, name="wk_sb")
    wk_v = ins["wk"].rearrange("p (l g c t m) -> p l g c t m",
                               l=2, g=4, c=2, t=9, m=128)
    for g in range(4):
        eng = nc.sync if g % 2 == 0 else nc.gpsimd
        eng.dma_start(wk_sb[:, 0:1, g:g + 1], wk_v[:, 0:1, g:g + 1])
    wv_sb = wpool.tile([128, 2, 4, 2, 9, 128], dt.float8e4, name="wv_sb")
    wv_v = ins["wv"].rearrange("p (l g c t m) -> p l g c t m",
                               l=2, g=4, c=2, t=9, m=128)
    for g in range(4):
        eng = nc.sync if g % 2 == 0 else nc.gpsimd
        eng.dma_start(wv_sb[:, 0:1, g:g + 1], wv_v[:, 0:1, g:g + 1])
    for g in range(4):
        eng = nc.sync if g % 2 == 0 else nc.gpsimd
        eng.dma_start(wk_sb[:, 1:2, g:g + 1], wk_v[:, 1:2, g:g + 1])
    for g in range(4):
        eng = nc.sync if g % 2 == 0 else nc.gpsimd
        eng.dma_start(wv_sb[:, 1:2, g:g + 1], wv_v[:, 1:2, g:g + 1])
    wo_sb = consts.tile([128, 4, 256], dt.bfloat16, name="wo_sb")
    nc.sync.dma_start(wo_sb, ins["wo"])
    bias_sb = consts.tile([128, 2], dt.float32, name="bias_sb")
    nc.sync.dma_start(bias_sb, ins["bias"])
    # identity matrix for PE transposes, built on-device
    ident_sb = consts.tile([128, 128], dt.bfloat16, name="ident_sb")
    nc.gpsimd.memset(ident_sb, 1.0)
    nc.gpsimd.affine_select(ident_sb, ident_sb, [[1, 128]],
                            mybir.AluOpType.is_equal, 0.0,
                            base=0, channel_multiplier=-1)

    # padded image view: [128, hl, c2, 34, 34]
    xv = xp_sb.rearrange("p l c (h w) -> p l c h w", h=HP, w=WP)
    # DoubleRow pass -> (weight hl, image hl): hi*hi + hi*lo + lo*hi
    PASSES = ((0, 0), (0, 1), (1, 0))

    # persistent conv outputs
    q_sb = [consts.tile([128, N], dt.bfloat16, name=f"q_sb{m}") for m in range(4)]
    k_sb = [consts.tile([128, N], dt.bfloat16, name=f"k_sb{m}") for m in range(4)]
    # vaug[jc][mi]: [128 pix, 2 heads, 64 v + 1 ones]; split per v-conv
    # unit so attention on early heads never waits late v-conv units.
    # v' = 16v (fp8 weight scaling), so the denominator column carries 16
    # too: reciprocal then yields 1/(16*s) and the gelu scale normalizes
    # both the 16x and the softmax sum at once.
    va_sb = [[consts.tile([128, 2, 65], dt.bfloat16, name=f"va{j}_{m}")
              for m in range(4)] for j in range(8)]
    for jc in range(8):
        for mi in range(4):
            nc.gpsimd.memset(va_sb[jc][mi][:, :, 64:65], 16.0)

    cpool = tc.alloc_tile_pool(name="cps", bufs=2, space="PSUM")
    epool = tc.alloc_tile_pool(name="eps", bufs=2, space="PSUM")
    # q conv gets its own 512-wide psum tiles (half the drain ops);
    # released before the v-transpose pool takes its banks (right stack)
    qpool = tc.alloc_tile_pool(name="qps", bufs=2, space="PSUM",
                               side="right")

    # ---------------- conv phase: step generators ----------------
    # Each conv "step" emits one PE matmul (plus any attached drains /
    # transposes). Dots tiles are interleaved ~1 per 3 conv steps so
    # the ACT exp stream never starves or stalls the PE.
    def q_unit(mc, nh):
        pe = qpool.tile([128, 512], dt.float32, name="qps", tag="qps")
        for half in range(2):
            y0 = 16 * nh + 8 * half
            sl = slice(half * 256, half * 256 + 256)
            for i, (lw, lx) in enumerate(PASSES):
                lhsT = qtap_sb[:, lw, :, mc * 128:(mc + 1) * 128]
                rhs = xv[:, lx, :, 1 + y0: 1 + y0 + 8, 1: 1 + 32]
                nc.tensor.matmul(pe[:, sl], lhsT, rhs,
                                 start=(i == 0), stop=(i == 2),
                                 perf_mode=mybir.MatmulPerfMode.DoubleRow)
                yield
        # one 512-wide drain per unit, alternating ACT/DVE
        dst = q_sb[mc][:, nh * 512:(nh + 1) * 512]
        if (mc + nh) % 2 == 0:
            nc.scalar.activation(dst, pe, AF.Copy)
        else:
            nc.vector.tensor_copy(dst, pe)

    def conv_steps():
        # q sub-units interleaved with k g0 sub-units: 4 q-subs then one
        # k-sub, so the small fast-filling q units never outrun their
        # drains while k work keeps the PE busy
        q_subs = [(mc, nh) for mc in range(4) for nh in range(2)]
        qi = 0
        for g in range(4):
            for nh in range(2):
                for half in range(2):
                    while g == 0 and qi < 2 * (nh * 2 + half + 1):
                        yield from q_unit(*q_subs[qi])
                        qi += 1
                    pe = cpool.tile([128, 256], dt.float32, name="cps",
                                    tag="cps")
                    y0 = 16 * nh + 8 * half
                    seq = [(t, lw, lx) for (lw, lx) in PASSES
                           for t in range(9)]
                    for i, (t, lw, lx) in enumerate(seq):
                        ky, kx = t // 3, t % 3
                        lhsT = wk_sb[:, lw, g, :, t, :]
                        rhs = xv[:, lx, :, ky + y0: ky + y0 + 8, kx: kx + 32]
                        nc.tensor.matmul(
                            pe, lhsT, rhs,
                            start=(i == 0), stop=(i == len(seq) - 1),
                            perf_mode=mybir.MatmulPerfMode.DoubleRow)
                        yield
                    co = nh * 512 + half * 256
                    nc.vector.tensor_copy(k_sb[g][:, co:co + 256], pe)
            k_ready[g] = True
            if g == 0:
                qpool.release()
        vtpool[0] = tc.alloc_tile_pool(name="vtp", bufs=1, space="PSUM",
                                       side="right")
        # v conv (weight stationary) + PE transpose into va.
        # Units are jc-pair sized (8 image rows -> 256 px) so each unit's
        # psum-drain -> transpose -> va-drain chain hides under the next
        # unit's matmuls; only the final unit's two transposes spill into
        # the first attention window (late_tps).
        for mi in range(4):
            for nh in range(2):
                for half in range(2):
                    pe = cpool.tile([128, 256], dt.float32, name="cps",
                                    tag="cps")
                    y0 = 16 * nh + 8 * half
                    seq = [(t, lw, lx) for (lw, lx) in PASSES
                           for t in range(9)]
                    for i, (t, lw, lx) in enumerate(seq):
                        ky, kx = t // 3, t % 3
                        lhsT = wv_sb[:, lw, mi, :, t, :]
                        rhs = xv[:, lx, :, ky + y0: ky + y0 + 8, kx: kx + 32]
                        nc.tensor.matmul(
                            pe, lhsT, rhs,
                            start=(i == 0), stop=(i == len(seq) - 1),
                            perf_mode=mybir.MatmulPerfMode.DoubleRow)
                        yield
                    vsb = vspool.tile([128, 256], dt.bfloat16, name="vsb",
                                      tag="vsb")
                    last_unit = (mi == 3 and nh == 1 and half == 1)
                    for sub in range(2):
                        jc = nh * 4 + half * 2 + sub
                        sl = slice(sub * 128, (sub + 1) * 128)
                        nc.vector.tensor_copy(vsb[:, sl], pe[:, sl])

                        def tp_step(jc=jc, sl=sl, mi=mi, vsb=vsb):
                            tp = vtpool[0].tile([128, 128], dt.bfloat16,
                                                name="vt", tag="vt")
                            nc.tensor.transpose(tp, vsb[:, sl], ident_sb)
                            nc.vector.tensor_copy(
                                va_sb[jc][mi][:, :, 0:64],
                                tp.rearrange("p (a b) -> p a b", a=2, b=64))
                        if last_unit:
                            late_tps.append(tp_step)
                        else:
                            tp_step()
                            yield

    k_ready = [False] * 4
    late_tps = []
    vtpool = [None]
    et_tiles = {h: [None] * 8 for h in range(HEADS)}
    et_pools = {}

    def dots_tile(h, jc):
        if h not in et_pools:
            et_pools[h] = tc.alloc_tile_pool(name=f"etp{h}", bufs=1,
                                             side="right")
        g, p = h // 2, h % 2
        ps, pe_ = 64 * p, 64 * p + 64
        pse = epool.tile([128, N], dt.float32, name="eps", tag="eps")
        for ic in range(2):
            lhsT = k_sb[g][ps:pe_, jc * 128:(jc + 1) * 128]
            rhs = q_sb[g][ps:pe_, ic * 512:(ic + 1) * 512]
            nc.tensor.matmul(pse[:, ic * 512:(ic + 1) * 512], lhsT, rhs,
                             start=True, stop=True)
        et = et_pools[h].tile([128, N], dt.bfloat16, name=f"et{h}_{jc}",
                              tag=f"et{h}_{jc}")
        nc.scalar.activation(et, pse, AF.Exp, scale=SCALE / 256.0)
        et_tiles[h][jc] = et

    # interleave: after the q conv and k g0, one dots tile per 3 conv steps
    cs = conv_steps()
    n_head_start = 48 + 108  # q (16*3) + k g0 (4*27)
    emitted = 0
    for _ in range(n_head_start):
        next(cs)
        emitted += 1
    dots_queue = [(h, jc) for h in range(HEADS) for jc in range(8)]
    dq = 0
    warm_sb = consts.tile([128, 1], dt.float32, name="warm_sb")
    conv_left = True
    while conv_left:
        if dq < len(dots_queue) and k_ready[dots_queue[dq][0] // 2]:
            h, jc = dots_queue[dq]
            dots_tile(h, jc)
            dq += 1
            if dq == len(dots_queue):
                # preload the Gelu table the moment the last exp is queued
                nc.scalar.activation(warm_sb, ident_sb[:, 0:1], AF.Gelu)
        for _ in range(12):
            try:
                next(cs)
                emitted += 1
            except StopIteration:
                conv_left = False
                break
    while dq < len(dots_queue):
        h, jc = dots_queue[dq]
        dots_tile(h, jc)
        dq += 1

    # ---------------- attention phase ----------------
    # vtpool/vspool stay alive: the last v-unit transposes run inside
    # the first attention window
    epool.release()
    cpool.release()
    wpool.release()

    gtpool = tc.alloc_tile_pool(name="gtpool", bufs=3)

    apool = tc.alloc_tile_pool(name="apool", bufs=3)
    rpool = tc.alloc_tile_pool(name="rpool", bufs=2)
    obpool = tc.alloc_tile_pool(name="obpool", bufs=4)

    popool = tc.alloc_tile_pool(name="pop", bufs=5, space="PSUM")
    gtppool = tc.alloc_tile_pool(name="gtp", bufs=1, space="PSUM")
    pfpool = None
    if True:

        gg_tiles = {}
        gt_tiles = {}

        def stage_attnv(ic):
            # GPSIMD cannot touch PSUM on hardware, so the po drain is the
            # gelu itself (ACT, scale = 1/denominator via DVE reciprocal)
            gg = apool.tile([128, 512], dt.bfloat16, name="gg", tag="gg")
            gg_tiles[ic] = gg
            r = rpool.tile([128, 8], dt.float32, name="r", tag="r")
            for h in range(HEADS):
                po = popool.tile([128, 65], dt.float32, name="po", tag="po")
                for jc in range(8):
                    nc.tensor.matmul(
                        po, et_tiles[h][jc][:, ic * 128:(ic + 1) * 128],
                        va_sb[jc][h // 2][:, h % 2, :],
                        start=(jc == 0), stop=(jc == 7))
                nc.vector.reciprocal(r[:, h:h + 1], po[:, 64:65])
                nc.scalar.activation(
                    gg[:, h * 64:(h + 1) * 64], po[:, 0:64],
                    AF.Gelu, scale=r[:, h:h + 1])
                if ic == 0 and late_tps and h < len(late_tps):
                    late_tps[h]()
                yield
            gt_tiles[ic] = gtpool.tile([128, 4, 128], dt.bfloat16,
                                       name="gt", tag="gt")

        def stage_transpose(ic):
            gg = gg_tiles[ic]
            for g in range(4):
                tp = gtppool.tile([128, 128], dt.bfloat16, name="gtp", tag="gtp")
                nc.tensor.transpose(tp, gg[:, g * 128:(g + 1) * 128], ident_sb)
                nc.vector.tensor_copy(gt_tiles[ic][:, g, :], tp)
                yield

        def stage_proj(ic):
            for co in range(2):
                pf = pfpool.tile([128, 128], dt.float32, name="pf", tag="pf")
                for g in range(4):
                    nc.tensor.matmul(
                        pf, wo_sb[:, g, co * 128:(co + 1) * 128],
                        gt_tiles[ic][:, g, :],
                        start=(g == 0), stop=(g == 3))
                ob = obpool.tile([128, 128], dt.float32, name="ob", tag="ob")
                if co == 0:
                    nc.scalar.activation(ob, pf, AF.Identity,
                                         bias=bias_sb[:, co:co + 1])
                else:
                    nc.vector.tensor_scalar_add(ob, pf, bias_sb[:, co:co + 1])
                eng = nc.sync if (ic + co) % 2 == 0 else nc.gpsimd
                eng.dma_start(
                    out_ap[co * 128:(co + 1) * 128,
                           ic * 128:(ic + 1) * 128], ob)
                yield

        # software pipeline: attnv(ic) | transpose(ic-1) | proj(ic-2)
        stages = {}
        for ic in range(10):
            if ic == 2:
                # last v-unit transposes (window 0) are done; swap the
                # vtp bank for the projection psum pool
                vtpool[0].release()
                pfpool = tc.alloc_tile_pool(name="pfp", bufs=2, space="PSUM")
            if ic < 8:
                stages[ic, "a"] = stage_attnv(ic)
            if 1 <= ic <= 8:
                stages[ic - 1, "t"] = stage_transpose(ic - 1)
            if ic >= 2:
                stages[ic - 2, "p"] = stage_proj(ic - 2)
            gens = [stages.get((ic, "a")), stages.get((ic - 1, "t")),
                    stages.get((ic - 2, "p"))]
            # round-robin drain: a-steps are big (8 matmuls), t/p small
            live = [g for g in gens if g is not None]
            while live:
                for g in list(live):
                    try:
                        next(g)
                    except StopIteration:
                        live.remove(g)

    pfpool.release()
    gtppool.release()
    popool.release()
    obpool.release()
    rpool.release()
    apool.release()
    gtpool.release()
    vspool.release()
    for h in reversed(range(HEADS)):
        et_pools[h].release()


def build_nc(repeat=1):
    nc = bacc.Bacc(trn_type="TRN2", target_bir_lowering=False, debug=False)
    ins = {
        "xp": nc.dram_tensor("xp", [128, 2 * 2 * HP * WP], dt.float8e4,
                             kind="ExternalInput").ap(),
        "wq": nc.dram_tensor("wq", [128, 2 * 2 * 512], dt.float8e4,
                             kind="ExternalInput").ap(),
        "wk": nc.dram_tensor("wk", [128, 2 * 4 * 2 * 9 * 128], dt.float8e4,
                             kind="ExternalInput").ap(),
        "wv": nc.dram_tensor("wv", [128, 2 * 4 * 2 * 9 * 128], dt.float8e4,
                             kind="ExternalInput").ap(),
        "wo": nc.dram_tensor("wo", [128, 4 * 256], dt.bfloat16,
                             kind="ExternalInput").ap(),
        "bias": nc.dram_tensor("bias", [128, 2], dt.float32,
                               kind="ExternalInput").ap(),
    }
    out_ap = nc.dram_tensor("out", [256, N], dt.float32,
                            kind="ExternalOutput").ap()
    with tile.TileContext(nc) as tc:
        for _ in range(repeat):
            with ExitStack() as ctx:
                tc._emit_ctx = ctx
                emit(tc, ins, out_ap)
    nc.compile()
    return nc


F8 = ml_dtypes.float8_e4m3
W_SCALE = 16.0


def _hi_lo(a):
    """f32 array -> (hi, lo) float8_e4m3 split with hi + lo ~= a."""
    hi = a.astype(F8)
    lo = (a - hi.astype(np.float32)).astype(F8)
    return hi, lo


def pack_weights(Wq, Wkv, Wout, bout):
    """Host-side packing of weights into the DRAM layouts the kernel expects."""
    q_hl = _hi_lo(Wq[:, :, 0, 0].T * W_SCALE)    # [256 cin, 512 cout] x2
    wq = np.stack([(a.reshape(2, 128, 512).transpose(1, 0, 2))
                   for a in q_hl], axis=1)       # [128, hl, c2, 512]
    wq = np.ascontiguousarray(wq).reshape(128, -1)

    def pack_conv3(Wpart):
        # Wpart: [512 cout, 256 cin, 3, 3] -> [128 p, hl, g, c2, t, 128]
        taps = np.stack([Wpart[:, :, t // 3, t % 3].T for t in range(9)])
        # taps: [9 t, 256 cin, 512 cout]
        hl = _hi_lo(taps * W_SCALE)
        arrs = []
        for a in hl:
            a = a.reshape(9, 2, 128, 4, 128)      # t, c2, p, g, m
            arrs.append(a.transpose(2, 3, 1, 0, 4))  # p, g, c2, t, m
        arr = np.stack(arrs, axis=1)              # p, hl, g, c2, t, m
        return np.ascontiguousarray(arr).reshape(128, -1)

    wk = pack_conv3(Wkv[0:INNER])
    wv = pack_conv3(Wkv[INNER:])
    wo = (Wout[:, :, 0, 0].T             # [512, 256]
          .reshape(4, 128, 256)
          .transpose(1, 0, 2)
          .reshape(128, 4 * 256).astype(BF16))
    bias = np.ascontiguousarray(bout.reshape(2, 128).T).astype(np.float32)
    return wq, wk, wv, wo, bias


def pack_x(xb):
    """One batch element [256, 32, 32] -> padded [128, hl, c2, 34*34] fp8."""
    xpad = np.zeros((C, HP, WP), np.float32)
    xpad[:, 1:33, 1:33] = xb
    hl = _hi_lo(xpad.reshape(2, 128, HP * WP).transpose(1, 0, 2))
    arr = np.stack(hl, axis=1)  # [128, hl, c2, 34*34]
    return np.ascontiguousarray(arr).reshape(128, -1)


_compiled = {}


def kernel(x, Wq, Wkv, Wout, bout, _trace=False, _tmpdir=None):
    x = np.asarray(x, np.float32)
    Wq = np.asarray(Wq, np.float32)
    Wkv = np.asarray(Wkv, np.float32)
    Wout = np.asarray(Wout, np.float32)
    bout = np.asarray(bout, np.float32)

    if "nc" not in _compiled:
        _compiled["nc"] = build_nc()
    nc = _compiled["nc"]

    wq, wk, wv, wo, bias = pack_weights(Wq, Wkv, Wout, bout)
    in_maps = []
    for b in range(NCORES):
        in_maps.append({
            "xp": pack_x(x[b]),
            "wq": wq, "wk": wk, "wv": wv, "wo": wo, "bias": bias,
        })

    res = run_bass_kernel_spmd(nc, in_maps, core_ids=list(range(NCORES)),
                               trace=_trace, tmpdir=_tmpdir)
    outs = [res.results[b]["out"].reshape(C, H, W) for b in range(NCORES)]
    full = np.stack(outs).astype(np.float32)
    if _trace:
        return full, res
    return full
